# revision 17
# baseline (speedup 1.0000x reference)
"""Trainium2 Bass kernel for nn_Attn: softmax(enc @ (W^T h)) over seq_len.

Math: energy = enc @ W^T + b; attn = energy @ h; out = softmax(attn).
Algebraically attn[s] = enc[s,:] . v + (b.h) with v = W^T h, and the (b.h)
term is constant across s so softmax cancels it. The device work is the
memory-bound part: streaming encoder_outputs once, sharded along seq_len
across 8 NeuronCores. The stream is sent as fp16 (host casts; softmax
rel-err stays ~5e-3, well inside tolerance), halving HBM traffic vs f32.

Layout: the host prepends one block of v replicated to 128 rows, so v
arrives as the stream's first tile (a separate small DMA on another
queue measured ~2us late - the DMA engines round-robin rings, so stream
traffic delays it). Each DMA group of n row-blocks puts n CONSECUTIVE
rows on each partition ("(p j) h -> p j h"), making the per-partition
DMA line n*2KiB contiguous (single-row lines are 2KiB in fp16 and
measured only ~269 GB/s vs ~330 for 8KiB). The dot with v is row-wise,
so the host undoes the row scramble when reassembling. Groups stay at
<=1MiB: 2MiB tiles coarsened the pipeline (compute waits on whole-tile
DMA completion) and measured slower end-to-end.

Per 128-row block the dot with v is either
  - TT path: DVE tensor_tensor multiply (2x 16-bit mode, ~0.69us) +
    row-sum on the Scalar engine (activation Copy + accum_out, ~1.41us;
    the accumulate path is 1 elem/cycle on every engine,
    dtype-independent), or
  - STT path: one DVE scalar_tensor_tensor (in0*1.0)*v with accum_out =
    row-sum (~1.3us, one pass, no Scalar work).
Blocks with b%8 in {2,5,7} take the STT path: the interleave keeps
Vector (~28us) and Scalar (~28us) both continuously busy just above the
~26us DMA stream; an all-TT or all-STT tail measurably extends the
drain. The final block is two half-width STTs so only ~0.7us of work
remains after the last bytes land.
"""
import numpy as np

S = 32768
H = 1024
N_CORES = 8
S_SHARD = S // N_CORES          # 4096 rows per core
P = 128                         # partitions
N_BLK = S_SHARD // P            # 32 row-blocks per core
# DMA groups over the 31 full enc blocks (block 31 is streamed as two
# H-halves at the end; the v-block tile precedes all of these)
GROUPS = [1, 1, 2, 4, 4, 4, 4, 4, 4, 2, 1]
STT_MOD = {2, 5, 7}             # b % 8 in this set -> STT path (blocks < 31)

_cache = {}


def _build():
    from concourse import bacc, mybir, tile

    nc = bacc.Bacc("TRN2", target_bir_lowering=False, debug=False,
                   num_devices=N_CORES)
    enc = nc.dram_tensor("enc", [S_SHARD + P, H], mybir.dt.float16,
                         kind="ExternalInput")
    # cols 0..30: blocks 0..30; cols 31,32: the two half-sums of block 31
    e_out = nc.dram_tensor("e_out", [P, N_BLK + 1], mybir.dt.float32,
                           kind="ExternalOutput")

    with tile.TileContext(nc) as tc:
        with tc.tile_pool(name="const", bufs=1) as cpool, \
             tc.tile_pool(name="stream", bufs=6) as spool, \
             tc.tile_pool(name="prod", bufs=4) as ppool:
            vt = cpool.tile([P, 1, H], mybir.dt.float16)
            nc.sync.dma_start(out=vt[:, 0, :], in_=enc.ap()[0:P, :])
            E = cpool.tile([P, N_BLK + 1], mybir.dt.float32)
            b0 = 0
            for n in GROUPS:
                t = spool.tile([P, n, H], mybir.dt.float16, tag=f"t{n}")
                rows = enc.ap()[(1 + b0) * P:(1 + b0 + n) * P, :]
                # partition p holds enc rows b0*128 + n*p + j, j<n
                nc.sync.dma_start(out=t[:],
                                  in_=rows.rearrange("(p j) h -> p j h", j=n))
                b = b0
                while b < b0 + n:
                    if b % 8 in STT_MOD:
                        o = ppool.tile([P, H], mybir.dt.float16, tag="so")
                        nc.vector.scalar_tensor_tensor(
                            out=o[:], in0=t[:, b - b0, :], scalar=1.0,
                            in1=vt[:, 0, :],
                            op0=mybir.AluOpType.mult,
                            op1=mybir.AluOpType.mult,
                            accum_out=E[:, b:b + 1])
                        b += 1
                    else:
                        # TT path: multiply on Vector, reduce on Scalar
                        w = 1
                        prod = ppool.tile([P, w, H], mybir.dt.float16,
                                          tag=f"prod{w}")
                        nc.vector.tensor_tensor(
                            out=prod[:], in0=t[:, b - b0:b - b0 + w, :],
                            in1=vt[:].broadcast_to((P, w, H)),
                            op=mybir.AluOpType.mult)
                        for k in range(w):
                            cp = ppool.tile([P, H], mybir.dt.float16,
                                            tag="cp")
                            nc.scalar.activation(
                                out=cp[:], in_=prod[:, k, :],
                                func=mybir.ActivationFunctionType.Copy,
                                accum_out=E[:, b + k:b + k + 1])
                        b += w
                b0 += n
                if b0 in (8, 16, 24):
                    k = b0 // 8 - 1
                    nc.sync.dma_start(
                        out=e_out.ap()[:, k * 8:(k + 1) * 8],
                        in_=E[:, k * 8:(k + 1) * 8])
            # final block as two half-width STTs: after its (late) bytes
            # land, only ~0.7us of work remains on the critical path
            HH = H // 2
            last = (1 + N_BLK - 1) * P
            th0 = spool.tile([P, HH], mybir.dt.float16, tag="th")
            th1 = spool.tile([P, HH], mybir.dt.float16, tag="th")
            nc.sync.dma_start(out=th0[:], in_=enc.ap()[last:last + P, 0:HH])
            nc.sync.dma_start(out=th1[:], in_=enc.ap()[last:last + P, HH:H])
            oh0 = ppool.tile([P, HH], mybir.dt.float16, tag="oh")
            nc.vector.scalar_tensor_tensor(
                out=oh0[:], in0=th0[:], scalar=1.0, in1=vt[:, 0, 0:HH],
                op0=mybir.AluOpType.mult, op1=mybir.AluOpType.mult,
                accum_out=E[:, N_BLK - 1:N_BLK])
            oh1 = ppool.tile([P, HH], mybir.dt.float16, tag="oh")
            nc.vector.scalar_tensor_tensor(
                out=oh1[:], in0=th1[:], scalar=1.0, in1=vt[:, 0, HH:H],
                op0=mybir.AluOpType.mult, op1=mybir.AluOpType.mult,
                accum_out=E[:, N_BLK:N_BLK + 1])
            nc.sync.dma_start(out=e_out.ap()[:, 24:N_BLK + 1],
                              in_=E[:, 24:N_BLK + 1])
    nc.compile()
    return nc


def _get_nc():
    if "nc" not in _cache:
        _cache["nc"] = _build()
    return _cache["nc"]


def kernel(hidden, encoder_outputs, W, b):
    from concourse import bass_utils

    nc = _get_nc()
    h = np.asarray(hidden, dtype=np.float32)[0]
    enc = np.ascontiguousarray(
        np.asarray(encoder_outputs, dtype=np.float32)[:, 0, :]
    ).astype(np.float16)
    v = (np.asarray(W, dtype=np.float32).T @ h).astype(np.float16)
    vrep = np.ascontiguousarray(np.broadcast_to(v, (P, H)))

    in_maps = [{"enc": np.concatenate(
        [vrep, enc[c * S_SHARD:(c + 1) * S_SHARD]])} for c in range(N_CORES)]
    res = bass_utils.run_bass_kernel_spmd(
        nc, in_maps, core_ids=list(range(N_CORES)),
        trace=_cache.get("trace", False))
    _cache["last_result"] = res

    # e_out column b holds, at partition p, the energy of shard row
    # 128*B_g + n_g*p + (b - B_g) for the group g containing block b;
    # within a group, eo[:, B:B+n].reshape(-1) is row order. Cols 31,32
    # are the two half-sums of block 31 (host adds them).
    shards = []
    for c in range(N_CORES):
        eo = res.results[c]["e_out"]
        e_shard = np.empty(S_SHARD, np.float32)
        B = 0
        for n in GROUPS:
            e_shard[128 * B:128 * (B + n)] = eo[:, B:B + n].reshape(-1)
            B += n
        e_shard[128 * (N_BLK - 1):] = eo[:, N_BLK - 1] + eo[:, N_BLK]
        shards.append(e_shard)
    e = np.concatenate(shards)
    e = e - e.max()
    p = np.exp(e)
    out = (p / p.sum()).astype(np.float32)
    return out[None, None, :]


# revision 19
# speedup vs baseline: 1.1431x; 1.1431x over previous
"""Trainium2 Bass kernel for nn_Attn: softmax(enc @ (W^T h)) over seq_len.

Math: energy = enc @ W^T + b; attn = energy @ h; out = softmax(attn).
Algebraically attn[s] = enc[s,:] . v + (b.h) with v = W^T h, and the (b.h)
term is constant across s so softmax cancels it. The device work is the
memory-bound part: streaming encoder_outputs once, sharded along seq_len
across 8 NeuronCores. The stream is sent as fp16 (host casts; softmax
rel-err stays ~5e-3, well inside tolerance), halving HBM traffic vs f32.

Layout: v is host-replicated to all 128 partitions (256 KiB) and DMA'd
once from the Scalar engine's queue, in parallel with the stream (the
gpsimd queue holds a ~3.4us framework DRAIN that delayed compute start
by ~4us). Each DMA group of n row-blocks puts n CONSECUTIVE rows on
each partition ("(p j) h -> p j h"), making the per-partition DMA line
n*2KiB contiguous (single-row lines are 2KiB in fp16 and measured only
~269 GB/s vs ~330 for 8KiB). The dot with v is row-wise, so the host
undoes the row scramble when reassembling. Groups stay at <=1MiB: 2MiB
tiles coarsened the pipeline (compute waits on whole-tile DMA
completion) and measured slower end-to-end.

Per 128-row block the dot with v is either
  - TT path: DVE tensor_tensor multiply (2x 16-bit mode, ~0.69us) +
    row-sum on the Scalar engine (activation Copy + accum_out, ~1.41us;
    the accumulate path is 1 elem/cycle on every engine,
    dtype-independent), or
  - STT path: one DVE scalar_tensor_tensor (in0*1.0)*v with accum_out =
    row-sum (~1.3us, one pass, no Scalar work).
Blocks with b%8 in {2,5,7} take the STT path: the interleave keeps
Vector (~28us) and Scalar (~28us) both continuously busy just above the
~26us DMA stream; an all-TT or all-STT tail measurably extends the
drain. The final block is two half-width STTs so only ~0.7us of work
remains after the last bytes land.
"""
import numpy as np

S = 32768
H = 1024
N_CORES = 8
S_SHARD = S // N_CORES          # 4096 rows per core
P = 128                         # partitions
N_BLK = S_SHARD // P            # 32 row-blocks per core
# DMA groups over the 31 full enc blocks (block 31 is streamed as two
# H-halves at the end; the v-block tile precedes all of these)
GROUPS = [1, 1, 2, 4, 4, 4, 4, 4, 4, 2, 1]
STT_MOD = {2, 5, 7}             # b % 8 in this set -> STT path (blocks < 31)

_cache = {}


def _build():
    from concourse import bacc, mybir, tile

    nc = bacc.Bacc("TRN2", target_bir_lowering=False, debug=False,
                   num_devices=N_CORES)
    enc = nc.dram_tensor("enc", [S_SHARD, H], mybir.dt.float16,
                         kind="ExternalInput")
    v_in = nc.dram_tensor("v_in", [P, H], mybir.dt.float16,
                          kind="ExternalInput")
    # cols 0..30: blocks 0..30; cols 31,32: the two half-sums of block 31
    e_out = nc.dram_tensor("e_out", [P, N_BLK + 1], mybir.dt.float32,
                           kind="ExternalOutput")

    with tile.TileContext(nc) as tc:
        with tc.tile_pool(name="const", bufs=1) as cpool, \
             tc.tile_pool(name="stream", bufs=6) as spool, \
             tc.tile_pool(name="prod", bufs=4) as ppool:
            vt = cpool.tile([P, 1, H], mybir.dt.float16)
            nc.scalar.dma_start(out=vt[:, 0, :], in_=v_in.ap())
            E = cpool.tile([P, N_BLK + 1], mybir.dt.float32)
            b0 = 0
            for n in GROUPS:
                t = spool.tile([P, n, H], mybir.dt.float16, tag=f"t{n}")
                rows = enc.ap()[b0 * P:(b0 + n) * P, :]
                # partition p holds rows b0*128 + n*p + j, j<n
                nc.sync.dma_start(out=t[:],
                                  in_=rows.rearrange("(p j) h -> p j h", j=n))
                b = b0
                while b < b0 + n:
                    if b % 8 in STT_MOD:
                        o = ppool.tile([P, H], mybir.dt.float16, tag="so")
                        nc.vector.scalar_tensor_tensor(
                            out=o[:], in0=t[:, b - b0, :], scalar=1.0,
                            in1=vt[:, 0, :],
                            op0=mybir.AluOpType.mult,
                            op1=mybir.AluOpType.mult,
                            accum_out=E[:, b:b + 1])
                        b += 1
                    else:
                        # run of TT blocks adjacent in this tile -> one
                        # wide multiply, then per-block Scalar reduces
                        w = 1
                        prod = ppool.tile([P, w, H], mybir.dt.float16,
                                          tag=f"prod{w}")
                        nc.vector.tensor_tensor(
                            out=prod[:], in0=t[:, b - b0:b - b0 + w, :],
                            in1=vt[:].broadcast_to((P, w, H)),
                            op=mybir.AluOpType.mult)
                        for k in range(w):
                            cp = ppool.tile([P, H], mybir.dt.float16,
                                            tag="cp")
                            nc.scalar.activation(
                                out=cp[:], in_=prod[:, k, :],
                                func=mybir.ActivationFunctionType.Copy,
                                accum_out=E[:, b + k:b + k + 1])
                        b += w
                b0 += n
                if b0 in (8, 16, 24):
                    k = b0 // 8 - 1
                    nc.sync.dma_start(
                        out=e_out.ap()[:, k * 8:(k + 1) * 8],
                        in_=E[:, k * 8:(k + 1) * 8])
            # final block as two half-width STTs: after its (late) bytes
            # land, only ~0.7us of work remains on the critical path
            HH = H // 2
            last = (N_BLK - 1) * P
            th0 = spool.tile([P, HH], mybir.dt.float16, tag="th")
            th1 = spool.tile([P, HH], mybir.dt.float16, tag="th")
            nc.sync.dma_start(out=th0[:], in_=enc.ap()[last:last + P, 0:HH])
            nc.sync.dma_start(out=th1[:], in_=enc.ap()[last:last + P, HH:H])
            oh0 = ppool.tile([P, HH], mybir.dt.float16, tag="oh")
            nc.vector.scalar_tensor_tensor(
                out=oh0[:], in0=th0[:], scalar=1.0, in1=vt[:, 0, 0:HH],
                op0=mybir.AluOpType.mult, op1=mybir.AluOpType.mult,
                accum_out=E[:, N_BLK - 1:N_BLK])
            oh1 = ppool.tile([P, HH], mybir.dt.float16, tag="oh")
            nc.vector.scalar_tensor_tensor(
                out=oh1[:], in0=th1[:], scalar=1.0, in1=vt[:, 0, HH:H],
                op0=mybir.AluOpType.mult, op1=mybir.AluOpType.mult,
                accum_out=E[:, N_BLK:N_BLK + 1])
            nc.sync.dma_start(out=e_out.ap()[:, 24:N_BLK + 1],
                              in_=E[:, 24:N_BLK + 1])
    nc.compile()
    return nc


def _get_nc():
    if "nc" not in _cache:
        _cache["nc"] = _build()
    return _cache["nc"]


def kernel(hidden, encoder_outputs, W, b):
    from concourse import bass_utils

    nc = _get_nc()
    h = np.asarray(hidden, dtype=np.float32)[0]
    enc = np.ascontiguousarray(
        np.asarray(encoder_outputs, dtype=np.float32)[:, 0, :]
    ).astype(np.float16)
    v = (np.asarray(W, dtype=np.float32).T @ h).astype(np.float16)
    vrep = np.ascontiguousarray(np.broadcast_to(v, (P, H)))

    in_maps = [{"enc": enc[c * S_SHARD:(c + 1) * S_SHARD],
                "v_in": vrep} for c in range(N_CORES)]
    res = bass_utils.run_bass_kernel_spmd(
        nc, in_maps, core_ids=list(range(N_CORES)),
        trace=_cache.get("trace", False))
    _cache["last_result"] = res

    # e_out column b holds, at partition p, the energy of shard row
    # 128*B_g + n_g*p + (b - B_g) for the group g containing block b;
    # within a group, eo[:, B:B+n].reshape(-1) is row order. Cols 31,32
    # are the two half-sums of block 31 (host adds them).
    shards = []
    for c in range(N_CORES):
        eo = res.results[c]["e_out"]
        e_shard = np.empty(S_SHARD, np.float32)
        B = 0
        for n in GROUPS:
            e_shard[128 * B:128 * (B + n)] = eo[:, B:B + n].reshape(-1)
            B += n
        e_shard[128 * (N_BLK - 1):] = eo[:, N_BLK - 1] + eo[:, N_BLK]
        shards.append(e_shard)
    e = np.concatenate(shards)
    e = e - e.max()
    p = np.exp(e)
    out = (p / p.sum()).astype(np.float32)
    return out[None, None, :]
